# revision 15
# baseline (speedup 1.0000x reference)
"""Trainium2 Bass kernel for BSplineLayer: y = BSpline(knots, coeffs, k=3)((x - min(x)) / (max(x) - min(x) + 1e-8)).

The reference clips the de Boor interval index to [k, n-1] = [3, 3], so the
layer is a single cubic P_unit(z) evaluated at z = s*x + b where s, b come
from the global min/max.  Host-side we factor the cubic exactly (every real
cubic has a real root r):

    P_unit(z) = a3 * (z - r) * ((z + U/2)^2 + D)

so the device evaluates, per element, just three streaming passes:

    sqp = Square(s*x + (b + U/2))        # ACT, scale/bias are runtime APs
    t1  = K1*x + K2                      # a3*(z - r); DVE TS @2x or ACT affine
    y   = (D + sqp) * t1                 # DVE scalar_tensor_tensor

K1 = a3*s, K2 = a3*(b - r); r, U/2, D, a3 are compile-time immediates
(program cache is keyed on knots/coeffs bytes).  The t1 pass alternates
between ACT and DVE so both engines stay under the store-DMA budget.

Phase 1 uses tensor_tensor_reduce (op0=op1=max / min) over PAIRS of tiles:
one DVE pass scans two tensors (2 elem/cycle), so local min+max costs ~35us
of DVE instead of ~70us and hides entirely under the ~47us of HBM loads.
The last tile pair is processed in half/quarter pieces to shrink the
after-last-load tail.  A dependency-free warm-up collective absorbs the
ncfw setup cost; the real AllReduce(max) exchanges (gmax, -gmin).
"""

import sys

sys.path.insert(0, "/opt/trn_rl_repo")

import numpy as np

N_CORES = 8
ROWS, COLS = 8192, 4096
R_CORE = ROWS // N_CORES          # 1024 rows per core
P = 128                           # SBUF partitions
N_TILES = R_CORE // P             # 8 tiles of [128, 4096] per core
CHUNK = 4096                      # phase-2 free-dim chunk (one tile)
DEGREE = 3

_CACHE = {}


def _expand_cubic(knots: np.ndarray, coeffs: np.ndarray) -> np.ndarray:
    """Expand de Boor at interval m=3 into monomial coeffs [a0, a1, a2, a3] (float64)."""
    t = np.asarray(knots, dtype=np.float64)
    c = np.asarray(coeffs, dtype=np.float64)
    k = DEGREE
    m = k  # reference clips searchsorted result to [k, n-1] with n-1 == k
    pm = np.polynomial.polynomial
    d = [np.array([c[m - k + j]], dtype=np.float64) for j in range(k + 1)]
    for r in range(1, k + 1):
        for j in range(k, r - 1, -1):
            tl = t[m - k + j]
            tr = t[m + j + 1 - r]
            inv = 1.0 / (tr - tl)
            alpha = np.array([-tl * inv, inv])
            one_m = np.array([1.0 + tl * inv, -inv])
            d[j] = pm.polyadd(pm.polymul(one_m, d[j - 1]), pm.polymul(alpha, d[j]))
    a = np.zeros(4, dtype=np.float64)
    a[: len(d[k])] = d[k]
    return a


def _factor_cubic(a: np.ndarray):
    """P(z) = a3*(z - r)*((z + U/2)^2 + D) with real r, U, D (float64)."""
    a3 = a[3] if abs(a[3]) > 1e-30 else 1e-30
    roots = np.roots([a3, a[2], a[1], a[0]])
    # pick the real root (guaranteed >= 1); among real roots take the one
    # farthest from the z in [0, 1] working domain for conditioning
    real = [z for z in roots if abs(z.imag) <= 1e-9 * max(1.0, abs(z.real))]
    if not real:  # numerically-forced fallback: most-real root
        real = [min(roots, key=lambda z: abs(z.imag))]
    r = max(real, key=lambda z: abs(z.real - 0.5)).real
    others = [z for z in roots if z is not None]
    # U, V from deflation: z^2 + U z + V = poly / (z - r), computed stably
    # from the root set: remaining roots s1, s2 satisfy s1+s2=-U, s1*s2=V.
    rem = sorted(roots, key=lambda z: abs(z.real - r) + abs(z.imag))[1:]
    U = float(-(rem[0] + rem[1]).real)
    V = float((rem[0] * rem[1]).real)
    D = V - 0.25 * U * U
    return float(r), float(U), float(D), float(a3)


def _build_program(r: float, U: float, D: float, a3: float):
    import concourse.bass as bass
    import concourse.tile as tile
    from concourse import bacc, bass_isa, mybir

    dt = mybir.dt.float32
    OP = mybir.AluOpType
    AX = mybir.AxisListType
    AF = mybir.ActivationFunctionType
    FMAX = 3.4e38

    nc = bacc.Bacc("TRN2", target_bir_lowering=False, debug=False, num_devices=N_CORES)
    x_ext = nc.declare_dram_parameter("x", [R_CORE, COLS], dt, isOutput=False)
    y_ext = nc.declare_dram_parameter("y", [R_CORE, COLS], dt, isOutput=True)

    rsem = nc.alloc_semaphore("rsem")
    lsem = nc.alloc_semaphore("lsem")

    with tile.TileContext(nc) as tc:
        with (
            tc.tile_pool(name="xp", bufs=1) as xp,
            tc.tile_pool(name="sqp", bufs=2) as sqp_pool,
            tc.tile_pool(name="wp", bufs=2) as wp,
            tc.tile_pool(name="small", bufs=1) as small,
            tc.tile_pool(name="dram", bufs=1, space="DRAM") as dram,
        ):
            # Fire-and-forget ncfw warm-up collective: its execution brings up
            # the cross-core comm paths (notably D2D routing) that the manual
            # remote-DMA exchange below relies on.  Without it the cross-die
            # transfers straggle by milliseconds.  Nothing consumes its
            # result, so its (slow, ~60-90us) completion is off the critical
            # path.
            warm_in = dram.tile([1, 2], dt)
            warm_out = dram.tile([1, 2], dt)
            nc.gpsimd.collective_compute(
                "AllReduce", OP.max,
                replica_groups=[list(range(N_CORES))],
                ins=[warm_in[:].opt()], outs=[warm_out[:].opt()],
            )

            # Cross-core exchange buffers for the manual allgather (replaces
            # the ncfw AllReduce):  8 single-dest relative remote-DMA
            # broadcasts; instruction j sends par to tpb (me XOR j), landing
            # in that receiver's slots[:, j, :].  Across all senders each
            # receiver's slot j is filled by exactly one peer, so a local
            # max-reduce over slots completes the AllReduce(max).  Slots
            # j=1..7 are PREPPED here (desc-gen off the critical path); the
            # j=0 self-slot prep sits after partition_all_reduce so the
            # trigger inherits the RAW dependency on par.
            par = small.tile([P, 2], dt)
            slots = small.tile([P, N_CORES, 2], dt)
            for j in range(1, N_CORES):
                rdests = [None] * N_CORES
                rdests[j] = (0, j)
                nc.gpsimd.remote_dma_broadcast(
                    out_ap=slots[:, j, :], in_ap=par[:, 0:2],
                    remote_sem=rsem, local_sem=lsem, rdests=rdests)

            # ACT table warm-up: first activation triggers the table-set DMA
            # (~2.7us); run it on a [P,1] scratch during phase 1, off the
            # critical path.
            actwarm = small.tile([P, 2], dt)
            nc.vector.memset(actwarm[:, 0:1], 0.0)
            nc.scalar.activation(actwarm[:, 1:2], actwarm[:, 0:1], AF.Square,
                                 bias=0.0, scale=1.0)
            nc.scalar.activation(actwarm[:, 1:2], actwarm[:, 0:1], AF.Identity,
                                 bias=0.0, scale=1.0)

            # ---------------- phase 1: load + local min/max ----------------
            # DVE-only reduces: the GPSIMD Q7 cores must stay idle here --
            # the ncfw warm-up's comm bring-up runs on them, and any gp work
            # in this window starves it, delaying the manual exchange by tens
            # of us (measured).  DVE min+max per tile (2 x 4.42us) paces
            # phase 1 at ~71us of DVE, slightly past the ~58us load end.
            xts = []
            for t in range(N_TILES):
                xt = xp.tile([P, COLS], dt, tag=f"x{t}")
                xts.append(xt)

            def load(t, lo, hi):
                nc.sync.dma_start(out=xts[t][:, lo:hi],
                                  in_=x_ext[t * P:(t + 1) * P, lo:hi])

            for t in range(N_TILES):
                load(t, 0, COLS)

            rmin = small.tile([P, N_TILES], dt)
            rmax = small.tile([P, N_TILES], dt)
            for t in range(N_TILES):
                a = xts[t][:, :]
                nc.vector.tensor_reduce(rmax[:, t:t + 1], a, axis=AX.X,
                                        op=OP.max)
                nc.vector.tensor_reduce(rmin[:, t:t + 1], a, axis=AX.X,
                                        op=OP.min)

            pk = small.tile([P, 2], dt)
            nc.vector.tensor_reduce(pk[:, 0:1], rmax[:], axis=AX.X, op=OP.max)
            rmn = small.tile([P, 1], dt)
            nc.vector.tensor_reduce(rmn[:], rmin[:], axis=AX.X, op=OP.min)
            nc.vector.tensor_scalar_mul(pk[:, 1:2], rmn[:], -1.0)

            # cross-partition: every partition gets (local/global max, -min)
            nc.gpsimd.partition_all_reduce(par[:], pk[:], channels=P,
                                           reduce_op=bass_isa.ReduceOp.max)

            # cross-core allgather: self-slot prep (carries the par RAW dep
            # to the trigger), then fire all 8 prepped broadcasts.
            rdests0 = [None] * N_CORES
            rdests0[0] = (0, 0)
            nc.gpsimd.remote_dma_broadcast(
                out_ap=slots[:, 0, :], in_ap=par[:, 0:2],
                remote_sem=rsem, local_sem=lsem, rdests=rdests0)
            nc.gpsimd.trigger_dma(count=None)
            GG = small.tile([P, 2], dt)
            gwait = nc.vector.tensor_reduce(GG[:, 0:1], slots[:, :, 0:1],
                                            axis=AX.XY, op=OP.max)
            gwait.wait_op(rsem, 0, "sem-ge")
            nc.vector.tensor_reduce(GG[:, 1:2], slots[:, :, 1:2],
                                    axis=AX.XY, op=OP.max)

            # ------- device scalars: s, b and phase-2 coefficients -------
            # s = 1/(gmax + gnm + eps); b = gnm*s      (gnm = -gmin)
            # bias2 = b + U/2; K1 = a3*s; K2 = a3*(b - r)
            cf = small.tile([P, 6], dt)
            dd, s_, b_, bias2, K1, K2 = (cf[:, i:i + 1] for i in range(6))
            nc.vector.scalar_tensor_tensor(dd, GG[:, 0:1], 1e-8, GG[:, 1:2],
                                           op0=OP.add, op1=OP.add)
            nc.vector.reciprocal(s_, dd)
            nc.vector.tensor_tensor(b_, GG[:, 1:2], s_, op=OP.mult)
            nc.vector.tensor_scalar_add(bias2, b_, 0.5 * U)
            nc.vector.tensor_scalar_mul(K1, s_, a3)
            tb_ = small.tile([P, 1], dt)
            nc.vector.tensor_scalar_add(tb_, b_, -r)
            nc.vector.tensor_scalar_mul(K2, tb_, a3)

            # ACT-owned copy of (s, bias2, K1, K2): phase-2 ACT ops then wait
            # on at most one foreign semaphore.
            acoef = small.tile([P, 4], dt)
            nc.scalar.copy(acoef[:, 0:2], cf[:, 1:3])   # s_, b_ -> cols 0,1 (b_ unused)
            nc.scalar.copy(acoef[:, 2:4], cf[:, 3:5])   # bias2, K1 -> cols 2,3
            s_a = acoef[:, 0:1]
            bias2_a = acoef[:, 2:3]
            K1_a = acoef[:, 3:4]
            K2_d = K2  # DVE-side TS reads cf directly

            # ---------------- phase 2: evaluate + store ----------------
            # One chunk per tile ([128, 4096]).  Per chunk: ACT Square pass
            # (measured 3.7us), gp t1 affine (3.76us, line-rate), DVE
            # combining STT (4.33us) in place over x, then the store.  Each
            # engine owns exactly one pass, totalling 30-35us each -- well
            # under the 47us of store DMA, so phase 2 is store-bound.
            for ci in range(N_TILES):
                xc = xts[ci][:, :]
                sq = sqp_pool.tile([P, CHUNK], dt, tag="sq")
                nc.scalar.activation(sq[:], xc, AF.Square,
                                     bias=bias2_a, scale=s_a)
                t1 = wp.tile([P, CHUNK], dt, tag="t1")
                nc.gpsimd.tensor_scalar(t1[:], xc, K1_a, K2_d,
                                        op0=OP.mult, op1=OP.add)
                nc.vector.scalar_tensor_tensor(xc, sq[:], float(D), t1[:],
                                               op0=OP.add, op1=OP.mult)
                nc.sync.dma_start(out=y_ext[ci * P:(ci + 1) * P, :], in_=xc)

    # Raise the remote-arrival gate AFTER scheduling: each of the 8 arrivals
    # adds 16/8 = 2 to rsem.  The single-core scheduling sim cannot model
    # cross-core increments and would deadlock on >= 16 at trace time.
    bass._bass_rust.wait_op(gwait.ins, rsem, 16, "sem-ge", False)
    nc.compile()
    return nc


def kernel(x: np.ndarray, knots: np.ndarray, coeffs: np.ndarray) -> np.ndarray:
    from concourse.bass_utils import run_bass_kernel_spmd

    x = np.ascontiguousarray(np.asarray(x, dtype=np.float32))
    assert x.shape == (ROWS, COLS), x.shape

    a = _expand_cubic(knots, coeffs)
    r, U, D, a3 = _factor_cubic(a)

    key = (np.asarray(knots, np.float32).tobytes(),
           np.asarray(coeffs, np.float32).tobytes())
    if _CACHE.get("key") != key:
        _CACHE["nc"] = _build_program(r, U, D, a3)
        _CACHE["key"] = key
    nc = _CACHE["nc"]

    shards = [x[i * R_CORE:(i + 1) * R_CORE] for i in range(N_CORES)]
    in_maps = [{"x": s} for s in shards]

    import os
    trace = bool(int(os.environ.get("KERNEL_TRACE", "0")))
    res = run_bass_kernel_spmd(nc, in_maps, core_ids=list(range(N_CORES)),
                               trace=trace)
    if trace and res.exec_time_ns is not None:
        print(f"HW exec time: {res.exec_time_ns} ns")
        _CACHE["last_exec_time_ns"] = res.exec_time_ns
        _CACHE["last_trace"] = res.instructions_and_trace

    out = np.empty((ROWS, COLS), dtype=np.float32)
    for i in range(N_CORES):
        out[i * R_CORE:(i + 1) * R_CORE] = res.results[i]["y"]
    return out


# revision 16
# speedup vs baseline: 1.7109x; 1.7109x over previous
"""Trainium2 Bass kernel for BSplineLayer: y = BSpline(knots, coeffs, k=3)((x - min(x)) / (max(x) - min(x) + 1e-8)).

The reference clips the de Boor interval index to [k, n-1] = [3, 3], so the
layer is a single cubic P_unit(z) evaluated at z = s*x + b where s, b come
from the global min/max.  Host-side we factor the cubic exactly (every real
cubic has a real root r):

    P_unit(z) = a3 * (z - r) * ((z + U/2)^2 + D)

so the device evaluates, per element, just three streaming passes:

    sqp = Square(s*x + (b + U/2))        # ACT, scale/bias are runtime APs
    t1  = K1*x + K2                      # a3*(z - r); DVE TS @2x or ACT affine
    y   = (D + sqp) * t1                 # DVE scalar_tensor_tensor

K1 = a3*s, K2 = a3*(b - r); r, U/2, D, a3 are compile-time immediates
(program cache is keyed on knots/coeffs bytes).  The t1 pass alternates
ACT/DVE so both engines sit at ~44us, under the ~47us of store DMA.

Phase 1 keeps the GPSIMD Q7 cores idle on purpose: the ncfw warm-up
collective's lazy comm bring-up runs on them, and any Q7 work in this window
delays the real AllReduce by tens of us (measured).  DVE alone scans for
min+max (2 x 4.33us per [128,4096] tile, ~71us total) while tiles stream in;
x stays SBUF-resident (16 MiB) so HBM traffic is one read + one write.
"""

import sys

sys.path.insert(0, "/opt/trn_rl_repo")

import numpy as np

N_CORES = 8
ROWS, COLS = 8192, 4096
R_CORE = ROWS // N_CORES          # 1024 rows per core
P = 128                           # SBUF partitions
N_TILES = R_CORE // P             # 8 tiles of [128, 4096] per core
CHUNK = 4096                      # phase-2 free-dim chunk (one tile)
DEGREE = 3

_CACHE = {}


def _expand_cubic(knots: np.ndarray, coeffs: np.ndarray) -> np.ndarray:
    """Expand de Boor at interval m=3 into monomial coeffs [a0, a1, a2, a3] (float64)."""
    t = np.asarray(knots, dtype=np.float64)
    c = np.asarray(coeffs, dtype=np.float64)
    k = DEGREE
    m = k  # reference clips searchsorted result to [k, n-1] with n-1 == k
    pm = np.polynomial.polynomial
    d = [np.array([c[m - k + j]], dtype=np.float64) for j in range(k + 1)]
    for r in range(1, k + 1):
        for j in range(k, r - 1, -1):
            tl = t[m - k + j]
            tr = t[m + j + 1 - r]
            inv = 1.0 / (tr - tl)
            alpha = np.array([-tl * inv, inv])
            one_m = np.array([1.0 + tl * inv, -inv])
            d[j] = pm.polyadd(pm.polymul(one_m, d[j - 1]), pm.polymul(alpha, d[j]))
    a = np.zeros(4, dtype=np.float64)
    a[: len(d[k])] = d[k]
    return a


def _factor_cubic(a: np.ndarray):
    """P(z) = a3*(z - r)*((z + U/2)^2 + D) with real r, U, D (float64)."""
    a3 = a[3] if abs(a[3]) > 1e-30 else 1e-30
    roots = np.roots([a3, a[2], a[1], a[0]])
    # pick the real root (guaranteed >= 1); among real roots take the one
    # farthest from the z in [0, 1] working domain for conditioning
    real = [z for z in roots if abs(z.imag) <= 1e-9 * max(1.0, abs(z.real))]
    if not real:  # numerically-forced fallback: most-real root
        real = [min(roots, key=lambda z: abs(z.imag))]
    r = max(real, key=lambda z: abs(z.real - 0.5)).real
    rem = sorted(roots, key=lambda z: abs(z.real - r) + abs(z.imag))[1:]
    U = float(-(rem[0] + rem[1]).real)
    V = float((rem[0] * rem[1]).real)
    D = V - 0.25 * U * U
    return float(r), float(U), float(D), float(a3)


def _build_program(r: float, U: float, D: float, a3: float):
    import concourse.bass as bass
    import concourse.tile as tile
    from concourse import bacc, bass_isa, mybir

    dt = mybir.dt.float32
    OP = mybir.AluOpType
    AX = mybir.AxisListType
    AF = mybir.ActivationFunctionType

    nc = bacc.Bacc("TRN2", target_bir_lowering=False, debug=False, num_devices=N_CORES)
    x_ext = nc.declare_dram_parameter("x", [R_CORE, COLS], dt, isOutput=False)
    y_ext = nc.declare_dram_parameter("y", [R_CORE, COLS], dt, isOutput=True)

    with tile.TileContext(nc) as tc:
        with (
            tc.tile_pool(name="xp", bufs=1) as xp,
            tc.tile_pool(name="sqp", bufs=2) as sqp_pool,
            tc.tile_pool(name="wp", bufs=2) as wp,
            tc.tile_pool(name="small", bufs=1) as small,
            tc.tile_pool(name="dram", bufs=1, space="DRAM") as dram,
        ):
            # Warm the collective path (ncfw queue/ring setup + core-skew
            # sync) concurrently with the phase-1 loads so the real AllReduce
            # is cheap.  Gathers an uninitialized DRAM word on purpose: zero
            # dependencies means the gpsimd stream enqueues it immediately.
            warm_in = dram.tile([1, 2], dt)
            warm_out = dram.tile([1, 2], dt)
            nc.gpsimd.collective_compute(
                "AllReduce", OP.max,
                replica_groups=[list(range(N_CORES))],
                ins=[warm_in[:].opt()], outs=[warm_out[:].opt()],
            )

            # ACT table warm-up: the first activation triggers the table-set
            # DMA (~2.7us); run it on a [P,1] scratch during phase 1, off the
            # post-collective critical path.
            actwarm = small.tile([P, 2], dt)
            nc.vector.memset(actwarm[:, 0:1], 0.0)
            nc.scalar.activation(actwarm[:, 1:2], actwarm[:, 0:1], AF.Square,
                                 bias=0.0, scale=1.0)
            nc.scalar.activation(actwarm[:, 1:2], actwarm[:, 0:1], AF.Identity,
                                 bias=0.0, scale=1.0)

            # ---------------- phase 1: load + local min/max ----------------
            # DVE-only reduces (the Q7s must stay idle -- see module doc).
            xts = []
            for t in range(N_TILES):
                xt = xp.tile([P, COLS], dt, tag=f"x{t}")
                xts.append(xt)
                nc.sync.dma_start(out=xt[:],
                                  in_=x_ext[t * P:(t + 1) * P, :])

            rmin = small.tile([P, N_TILES], dt)
            rmax = small.tile([P, N_TILES], dt)
            for t in range(N_TILES):
                nc.vector.tensor_reduce(rmax[:, t:t + 1], xts[t][:, :],
                                        axis=AX.X, op=OP.max)
                nc.vector.tensor_reduce(rmin[:, t:t + 1], xts[t][:, :],
                                        axis=AX.X, op=OP.min)

            pk = small.tile([P, 2], dt)
            nc.vector.tensor_reduce(pk[:, 0:1], rmax[:], axis=AX.X, op=OP.max)
            rmn = small.tile([P, 1], dt)
            nc.vector.tensor_reduce(rmn[:], rmin[:], axis=AX.X, op=OP.min)
            nc.vector.tensor_scalar_mul(pk[:, 1:2], rmn[:], -1.0)

            # cross-partition: every partition gets (local_max, -local_min)
            par = small.tile([P, 2], dt)
            nc.gpsimd.partition_all_reduce(par[:], pk[:], channels=P,
                                           reduce_op=bass_isa.ReduceOp.max)

            # cross-core: AllReduce(max) of the pair
            cc_in = dram.tile([1, 2], dt)
            cc_out = dram.tile([1, 2], dt)
            nc.sync.dma_start(out=cc_in[:], in_=par[0:1, 0:2])
            nc.gpsimd.collective_compute(
                "AllReduce", OP.max,
                replica_groups=[list(range(N_CORES))],
                ins=[cc_in[:].opt()], outs=[cc_out[:].opt()],
            )
            GG = small.tile([P, 2], dt)
            nc.sync.dma_start(out=GG[:], in_=cc_out[:].partition_broadcast(P))

            # ------- device scalars: s, b and phase-2 coefficients -------
            # s = 1/(gmax + gnm + eps); b = gnm*s      (gnm = -gmin)
            # bias2 = b + U/2; K1 = a3*s; K2 = a3*(b - r)
            cf = small.tile([P, 6], dt)
            dd, s_, b_, bias2, K1, K2 = (cf[:, i:i + 1] for i in range(6))
            nc.vector.scalar_tensor_tensor(dd, GG[:, 0:1], 1e-8, GG[:, 1:2],
                                           op0=OP.add, op1=OP.add)
            nc.vector.reciprocal(s_, dd)
            nc.vector.tensor_tensor(b_, GG[:, 1:2], s_, op=OP.mult)
            nc.vector.tensor_scalar_add(bias2, b_, 0.5 * U)
            nc.vector.tensor_scalar_mul(K1, s_, a3)
            tb_ = small.tile([P, 1], dt)
            nc.vector.tensor_scalar_add(tb_, b_, -r)
            nc.vector.tensor_scalar_mul(K2, tb_, a3)

            # ACT-owned copy of (s, bias2, K1): phase-2 ACT ops then wait on
            # at most one foreign semaphore.
            acoef = small.tile([P, 4], dt)
            nc.scalar.copy(acoef[:, 0:2], cf[:, 1:3])   # s_, b_ (b_ unused)
            nc.scalar.copy(acoef[:, 2:4], cf[:, 3:5])   # bias2, K1
            s_a = acoef[:, 0:1]
            bias2_a = acoef[:, 2:3]
            K1_a = acoef[:, 3:4]
            K2_d = K2  # DVE-side TS reads cf directly

            # ---------------- phase 2: evaluate + store ----------------
            # One chunk per tile ([128, 4096]).  Per chunk: ACT Square
            # (~3.7us), t1 affine alternating ACT/DVE (balances both engines
            # at ~44us), the combining STT on DVE (4.33us) in place over x,
            # then the 2 MiB store.  Store DMA (8 x 5.9us) is the pacer.
            for ci in range(N_TILES):
                xc = xts[ci][:, :]
                sq = sqp_pool.tile([P, CHUNK], dt, tag="sq")
                nc.scalar.activation(sq[:], xc, AF.Square,
                                     bias=bias2_a, scale=s_a)
                t1 = wp.tile([P, CHUNK], dt, tag="t1")
                if ci % 2 == 1:
                    nc.scalar.activation(t1[:], xc, AF.Identity,
                                         bias=K2_d, scale=K1_a)
                else:
                    nc.vector.tensor_scalar(t1[:], xc, K1_a, K2_d,
                                            op0=OP.mult, op1=OP.add)
                nc.vector.scalar_tensor_tensor(xc, sq[:], float(D), t1[:],
                                               op0=OP.add, op1=OP.mult)
                nc.sync.dma_start(out=y_ext[ci * P:(ci + 1) * P, :], in_=xc)

    nc.compile()
    return nc


def kernel(x: np.ndarray, knots: np.ndarray, coeffs: np.ndarray) -> np.ndarray:
    from concourse.bass_utils import run_bass_kernel_spmd

    x = np.ascontiguousarray(np.asarray(x, dtype=np.float32))
    assert x.shape == (ROWS, COLS), x.shape

    a = _expand_cubic(knots, coeffs)
    r, U, D, a3 = _factor_cubic(a)

    key = (np.asarray(knots, np.float32).tobytes(),
           np.asarray(coeffs, np.float32).tobytes())
    if _CACHE.get("key") != key:
        _CACHE["nc"] = _build_program(r, U, D, a3)
        _CACHE["key"] = key
    nc = _CACHE["nc"]

    shards = [x[i * R_CORE:(i + 1) * R_CORE] for i in range(N_CORES)]
    in_maps = [{"x": s} for s in shards]

    import os
    trace = bool(int(os.environ.get("KERNEL_TRACE", "0")))
    res = run_bass_kernel_spmd(nc, in_maps, core_ids=list(range(N_CORES)),
                               trace=trace)
    if trace and res.exec_time_ns is not None:
        print(f"HW exec time: {res.exec_time_ns} ns")
        _CACHE["last_exec_time_ns"] = res.exec_time_ns
        _CACHE["last_trace"] = res.instructions_and_trace

    out = np.empty((ROWS, COLS), dtype=np.float32)
    for i in range(N_CORES):
        out[i * R_CORE:(i + 1) * R_CORE] = res.results[i]["y"]
    return out
